# revision 14
# baseline (speedup 1.0000x reference)
"""Bilateral filter v3: symmetric-pair sharing + TensorE shift-matrix accumulation.

Math: for each UNORDERED tap pair {+d, -d} (24 reps), the A-form contribution
shares one computation:
    d'(q) = xp(q+delta) - xp(q)     (computed at every padded center q)
    w'(q) = (2/sqrt(pi)) exp(-50 d'^2)   [ACT Derivative_Erf]
    t'(q) = w'(q) * d'(q)
  tap +d at p:  acc += k*t'(p),        den += k*w'(p)
  tap -d at p:  acc -= k*t'(p-delta),  den += k*w'(p-delta)   [exact: d' odd, w' even]
Row shifts (p-delta_y) ride the TensorE stationary (shifted scaled diagonal),
col shifts (delta_x) ride the matmul RHS free-dim offset.

Bands: tiles cover padded rows r0-3 .. r0+124 (128 partitions); the shifted
reads make only 125 output rows/band valid -> 9 overlapped bands per image
(r0 = 0,125,...,875,899; the last re-computes rows 899..999, harmless).
Band pairs pack 2 bands in the free dim; the 9th runs alone.

DVE: 2 ops/pair (sub, mul).  ACT: 1 derf/pair + PSUM evac (evac of den adds
the center tap's +1 via the activation bias).  TensorE: 16 chunk-matmuls/pair.
Prepass/pipeline/deferred-epilogue structure as v2.2.
"""

import sys

sys.path.insert(0, "/opt/trn_rl_repo")

import math
import numpy as np

SPATIAL_RADIUS = 3
COLOR_RADIUS = 0.1
INV_2C2 = 1.0 / (2.0 * COLOR_RADIUS**2)  # 50.0
INV_2R2 = 1.0 / (2.0 * float(SPATIAL_RADIUS) ** 2)  # 1/18

N_CORES = 8
_NC_CACHE = {}


def build_nc(n_img, H, W):
    import concourse.bacc as bacc
    import concourse.bass as bass
    import concourse.mybir as mybir
    from concourse.tile import TileContext

    f32 = mybir.dt.float32
    f16 = mybir.dt.float16
    R = SPATIAL_RADIUS  # 3
    P = 128
    Wd = W + 6  # d/w/t width: centers c' = -3..W+2
    Wt = W + 12  # T-tile width: real cols -6..W+5
    Wp = W + 14  # pad16 width: real cols -6..W+7 (even, covers T-odd j<=1035)
    CH = 512
    VR = P - 3  # valid output rows per band (125)

    # band starts (output rows r0..r0+124 per band)
    starts = list(range(0, H - VR + 1, VR))
    if starts[-1] != H - VR:
        starts.append(H - VR)
    bps = [tuple(starts[i : i + 2]) for i in range(0, len(starts) - 1, 2)]
    if len(starts) % 2 == 1:
        bps.append((starts[-1],))

    # pair representatives (delta_y, delta_x)
    reps = [(0, dx) for dx in (1, 2, 3)] + [(dy, dx) for dy in (1, 2, 3) for dx in range(-3, 4)]
    ds2_of = lambda dy, dx: dy * dy + dx * dx
    classes = sorted({ds2_of(*r) for r in reps})
    cls_idx = {v: i for i, v in enumerate(classes)}
    reps.sort(key=lambda r: (cls_idx[ds2_of(*r)], r))
    kval = {v: (math.sqrt(math.pi) / 2.0) * math.exp(-v * INV_2R2) for v in classes}

    nc = bacc.Bacc(None, target_bir_lowering=False)
    x = nc.declare_dram_parameter("x", [n_img * H, W], f32, isOutput=False)
    y = nc.declare_dram_parameter("y", [n_img * H, W], f32, isOutput=True)

    with TileContext(nc) as tc:
        with (
            tc.tile_pool(name="consts", bufs=1) as cpool,
            tc.tile_pool(name="drampool", bufs=1, space="DRAM") as dpool,
            tc.tile_pool(name="prepool", bufs=2) as ppool,
            tc.tile_pool(name="bandpool", bufs=2) as bpool,
            tc.tile_pool(name="workpool", bufs=1) as wpool,
            tc.tile_pool(name="psumpool", bufs=1, space="PSUM") as pspool,
        ):
            # ---- stationaries: shifted scaled diagonals ----
            def diag(tag, off, fill):
                # nonzero at k == m + off  (k partition, m free)
                t_id = cpool.tile([P, P], f16, tag=tag)
                nc.gpsimd.memset(t_id[:, :], 0.0)
                nc.gpsimd.affine_select(
                    out=t_id[:, :], in_=t_id[:, :],
                    compare_op=mybir.AluOpType.not_equal,
                    fill=fill, base=-off, pattern=[[-1, P]], channel_multiplier=1,
                )
                return t_id

            id_plus = {v: diag(f"p{v}", 3, kval[v]) for v in classes}
            id_macc = {}
            id_mden = {}
            for v in classes:
                for dy in sorted({r[0] for r in reps if ds2_of(*r) == v}):
                    id_macc[(v, dy)] = diag(f"a{v}_{dy}", 3 - dy, -kval[v])
                    id_mden[(v, dy)] = diag(f"d{v}_{dy}", 3 - dy, kval[v])
            # delta_x == 0 pairs: both terms read the same RHS columns, so the
            # two diagonals merge into ONE stationary -> 1 matmul per target
            id_m2acc = {}
            id_m2den = {}
            for dy, dx in reps:
                if dx == 0:
                    v = ds2_of(dy, dx)
                    ta = diag(f"ma{dy}", 3, kval[v])
                    nc.gpsimd.affine_select(
                        out=ta[:, :], in_=ta[:, :],
                        compare_op=mybir.AluOpType.not_equal,
                        fill=-kval[v], base=-(3 - dy), pattern=[[-1, P]], channel_multiplier=1,
                    )
                    id_m2acc[dy] = ta
                    td = diag(f"md{dy}", 3, kval[v])
                    nc.gpsimd.affine_select(
                        out=td[:, :], in_=td[:, :],
                        compare_op=mybir.AluOpType.not_equal,
                        fill=kval[v], base=-(3 - dy), pattern=[[-1, P]], channel_multiplier=1,
                    )
                    id_m2den[dy] = td

            def emit_prepass(gb, buf, fast=False):
                """Build pad16 (rows -3..H+2, cols -6..W+7). Normally GpSimd-only
                (overlaps band compute); `fast` spreads the first image's
                prepass over the still-idle ACT/DVE engines + sync queue."""
                pad = dpool.tile([H + 2 * R, Wp], f16, tag=f"pad{buf}")
                dma = nc.sync.dma_start if fast else nc.gpsimd.dma_start
                loaders = [nc.sync, nc.gpsimd, nc.scalar]
                nb = H // P
                # bands 0,1 + row pads first so the first band-pair's T-loads
                # can start while the rest of the image is still casting
                order = [0, 1, nb - 1] + list(range(2, nb - 1))
                for ki, k in enumerate(order):
                    xb = ppool.tile([P, W], f32, tag="xb")
                    pb = ppool.tile([P, Wp], f16, tag="pb")
                    ld = loaders[ki % 3].dma_start if fast else dma
                    ld(out=xb[:, :], in_=x[gb + k * P : gb + (k + 1) * P, :])
                    if fast:
                        nc.scalar.copy(pb[:, 6 : 6 + W], xb[:, :])
                    else:
                        nc.gpsimd.tensor_copy(pb[:, 6 : 6 + W], xb[:, :])
                    cols = list(range(6)) + list(range(W + 6, Wp))
                    for i, c in enumerate(cols):
                        src = 6 if c < 6 else W + 5
                        if fast:
                            if i % 2:
                                nc.vector.tensor_copy(pb[:, c : c + 1], pb[:, src : src + 1])
                            else:
                                nc.scalar.copy(pb[:, c : c + 1], pb[:, src : src + 1])
                        else:
                            nc.gpsimd.tensor_copy(pb[:, c : c + 1], pb[:, src : src + 1])
                    dma(out=pad[R + k * P : R + (k + 1) * P, :], in_=pb[:, :])
                    if k == 0:
                        for r in range(R):
                            dma(out=pad[r : r + 1, :], in_=pad[R : R + 1, :])
                    if k == nb - 1:
                        for r in range(R):
                            dma(
                                out=pad[H + R + r : H + R + r + 1, :],
                                in_=pad[H + R - 1 : H + R, :],
                            )
                return pad

            pending_epi = []

            def emit_image(gb, pad):
                def emit_bp(segs):
                    ns = len(segs)
                    FD = ns * Wd  # d/w/t free dim
                    FO = ns * W  # acc/den/out free dim

                    # T tiles: dy' = delta_y in 0..3, two parities
                    # T_even[dy][k, b, j] = xp(r0b-3+k+dy, j-6); T_odd: j-5
                    T = {}
                    for dy in range(4):
                        for par in range(2):
                            tt = bpool.tile([P, ns * Wt], f16, tag=f"T{dy}p{par}")
                            for b, r0 in enumerate(segs):
                                nc.sync.dma_start(
                                    out=tt[:, b * Wt : (b + 1) * Wt],
                                    in_=pad[r0 + dy : r0 + dy + P, par : par + Wt],
                                )
                            T[(dy, par)] = tt
                    # center rows for the final add: cen_epi[m, b, c] = xp(r0b+m, c)
                    cen_epi = bpool.tile([P, 2 * W], f16, tag="cen_epi")
                    for b, r0 in enumerate(segs):
                        nc.sync.dma_start(
                            out=cen_epi[:, b * W : (b + 1) * W],
                            in_=pad[r0 + 3 : r0 + 3 + P, 6 : 6 + W],
                        )

                    def tseg(tile_, off):
                        return tile_[:, :].rearrange("p (s c) -> p s c", c=Wt)[:, :, off : off + Wd]

                    cen = tseg(T[(0, 1)], 2)  # xp(q, c') at even offset

                    # always full-size (4 banks each) so the single-band bp
                    # reuses the same 8 PSUM banks instead of new ones
                    accP = pspool.tile([P, 2 * W], f32, tag="acc")
                    denP = pspool.tile([P, 2 * W], f32, tag="den")

                    for ri, (dy, dx) in enumerate(reps):
                        o = dx + 3
                        if o % 2 == 0:
                            s_ap = tseg(T[(dy, 0)], o)
                        else:
                            s_ap = tseg(T[(dy, 1)], o - 1)
                        # full-size tiles with a 5-deep rotation: lets DVE/ACT
                        # run several reps ahead so TensorE's fast (merged-rep)
                        # stretches don't starve
                        d = wpool.tile([P, 2 * Wd], f16, tag=f"d{ri % 5}")
                        w = wpool.tile([P, 2 * Wd], f16, tag=f"w{ri % 5}")
                        t = wpool.tile([P, 2 * Wd], f16, tag=f"t{ri % 5}")
                        d3 = d[:, 0:FD].rearrange("p (s c) -> p s c", c=Wd)
                        nc.vector.tensor_tensor(out=d3, in0=s_ap, in1=cen, op=mybir.AluOpType.subtract)
                        nc.scalar.activation(
                            w[:, 0:FD], d[:, 0:FD],
                            mybir.ActivationFunctionType.Derivative_Erf,
                            scale=math.sqrt(INV_2C2),
                        )
                        nc.vector.tensor_tensor(out=t[:, 0:FD], in0=w[:, 0:FD], in1=d[:, 0:FD], op=mybir.AluOpType.mult)

                        v = ds2_of(dy, dx)
                        first = ri == 0
                        last = ri == len(reps) - 1
                        chunks = [(b * W + j * CH, b * Wd + j * CH) for b in range(ns) for j in range(W // CH)]
                        if dx == 0:
                            # merged two-diagonal stationaries: 1 matmul/target
                            ma, md = id_m2acc[dy], id_m2den[dy]
                            for oc, ub in chunks:
                                up = ub + 3
                                nc.tensor.matmul(
                                    accP[:, oc : oc + CH], ma[:, :], t[:, up : up + CH],
                                    start=first, stop=last,
                                )
                                nc.tensor.matmul(
                                    denP[:, oc : oc + CH], md[:, :], w[:, up : up + CH],
                                    start=first, stop=last,
                                )
                        else:
                            lp, la, ld = id_plus[v], id_macc[(v, dy)], id_mden[(v, dy)]
                            # group by stationary: lp (8 mms), la (4), ld (4)
                            for oc, ub in chunks:
                                up = ub + 3
                                nc.tensor.matmul(
                                    accP[:, oc : oc + CH], lp[:, :], t[:, up : up + CH],
                                    start=first, stop=False,
                                )
                                nc.tensor.matmul(
                                    denP[:, oc : oc + CH], lp[:, :], w[:, up : up + CH],
                                    start=first, stop=False,
                                )
                            for oc, ub in chunks:
                                um = ub + 3 - dx
                                nc.tensor.matmul(
                                    accP[:, oc : oc + CH], la[:, :], t[:, um : um + CH],
                                    start=False, stop=last,
                                )
                            for oc, ub in chunks:
                                um = ub + 3 - dx
                                nc.tensor.matmul(
                                    denP[:, oc : oc + CH], ld[:, :], w[:, um : um + CH],
                                    start=False, stop=last,
                                )
                        if ri == 2 and pending_epi:
                            pending_epi.pop()()

                    def epilogue(accP=accP, denP=denP, cen_epi=cen_epi, segs=segs, FO=FO):
                        accS = wpool.tile([P, 2 * W], f32, tag="accS")
                        denS = wpool.tile([P, 2 * W], f32, tag="denS")
                        nc.scalar.copy(accS[:, 0:FO], accP[:, 0:FO])
                        # den evac + the center tap's +1, fused into the bias
                        nc.scalar.activation(
                            denS[:, 0:FO], denP[:, 0:FO],
                            mybir.ActivationFunctionType.Copy, bias=1.0,
                        )
                        rcp = wpool.tile([P, 2 * W], f32, tag="rcp")
                        scr = wpool.tile([P, 2 * W], f32, tag="scr")
                        nc.vector.reciprocal_approx_accurate(rcp[:, 0:FO], denS[:, 0:FO], scr[:, 0:FO])
                        nc.vector.tensor_tensor(out=scr[:, 0:FO], in0=accS[:, 0:FO], in1=rcp[:, 0:FO], op=mybir.AluOpType.mult)
                        nc.vector.tensor_tensor(out=scr[:, 0:FO], in0=scr[:, 0:FO], in1=cen_epi[:, 0:FO], op=mybir.AluOpType.add)
                        for b, r0 in enumerate(segs):
                            nc.sync.dma_start(
                                out=y[gb + r0 : gb + r0 + VR, :],
                                in_=scr[0:VR, b * W : (b + 1) * W],
                            )

                    pending_epi.append(epilogue)

                for segs in bps:
                    emit_bp(segs)

            pads = {0: emit_prepass(0, "A", fast=True)}
            if n_img > 1:
                pads[1] = emit_prepass(H, "B")
            for i in range(n_img):
                emit_image(i * H, pads.pop(i))
                nxt = i + 2
                if nxt < n_img:
                    pads[nxt] = emit_prepass(nxt * H, "AB"[nxt % 2])
            while pending_epi:
                pending_epi.pop()()

    nc.finalize()
    return nc


def _get_nc(n_img, H, W):
    key = (n_img, H, W)
    if key not in _NC_CACHE:
        _NC_CACHE[key] = build_nc(n_img, H, W)
    return _NC_CACHE[key]


def run_sharded(flat, n_img_per_core, H, W, trace=False):
    from concourse.bass_utils import run_bass_kernel_spmd

    nc = _get_nc(n_img_per_core, H, W)
    in_maps = [
        {
            "x": np.ascontiguousarray(
                flat[c * n_img_per_core : (c + 1) * n_img_per_core].reshape(n_img_per_core * H, W)
            )
        }
        for c in range(N_CORES)
    ]
    res = run_bass_kernel_spmd(nc, in_maps, core_ids=list(range(N_CORES)), trace=trace)
    out = np.stack([res.results[c]["y"].reshape(n_img_per_core, H, W) for c in range(N_CORES)])
    return out.reshape(N_CORES * n_img_per_core, H, W), res


def kernel(input_tensor: np.ndarray) -> np.ndarray:
    input_tensor = np.asarray(input_tensor, dtype=np.float32)
    B, C, H, W = input_tensor.shape
    flat = input_tensor.reshape(B * C, H, W)
    assert (B * C) % N_CORES == 0
    out, _ = run_sharded(flat, (B * C) // N_CORES, H, W)
    return out.reshape(B, C, H, W)
